# revision 2
# baseline (speedup 1.0000x reference)
"""Trainium2 Bass kernel for nn_BertWordPair (sparse_attention) — v2.

y = x @ W1 + b1 -> (q_tok, q_utt, k_tok, k_utt) per channel c; RoPE with
block-sign structure from seg_ids; logits [B, S, S, 3] = selected-variant
bilinear forms summed over the two groups.

Strategy (8 NeuronCores), v2:
  - 2x2 grid per batch: core (b, rh, ch) owns rows rh*1024..+1024 and cols
    ch*1024..+1024.  Stage-A computes q-features for the core's rows and
    k-features for the core's cols only (147K PE cycles vs 184K for 1x4).
  - bf16 datapath except PSUM accumulation: W/x/tables load bf16, stage-A
    PSUM (fp32) is bias-added + cast to bf16 on the Activation engine, all
    RoPE elementwise runs as bf16 tensor_tensor (2x DVE mode), stage-B
    matmuls take bf16 operands, output written bf16 and upcast on host.
  - Variant selection (pp / q_neg.k_pos / q_pos.k_neg) reduces to signs
    sigma_q(row-half, col-block) / sigma_k(row-half, col-block).  The signs
    are folded into per-variant HOST-built sin tables so the SPMD program
    stays uniform and sign application costs nothing:
      K side: sin_v[h][:, col] = sin[:, col] * sigma_k(h, blk(col))
      Q side: sin_v[nb][:, row] = sin[:, row] * sigma_q(half(row), nb)
    K_eff^h_e = y_e*cos - y_o*sin_v[h]   (plain TT sub/add, no scalars)
"""
import sys
sys.path.insert(0, '/opt/trn_rl_repo')

import numpy as np
import ml_dtypes

BF16 = ml_dtypes.bfloat16

B, S, H, C = 2, 2048, 768, 3
DG = 256             # rope dim per group (tok / utt)
D2 = 512             # feature dim per channel (tok 256 + utt 256)
N_CORES = 8
RROWS = 1024         # rows per core
CCOLS = 1024         # cols per core
BLK = 512            # sigma granularity (col block / row half)
NB = CCOLS // BLK    # 2
NH = RROWS // BLK    # 2
KH = H // 128        # 6 contraction tiles for dense1
FT = 12              # feature tiles per side (q or k): ft = c*4 + g*2 + p
MT = RROWS // 128    # 8 row tiles per core


def _variant(s, t):
    # 0=PP, 1=NP (q_neg*k_pos), 2=PN (q_pos*k_neg)
    if s >= 1 and t > s:
        return 1
    if t >= 1 and s > t:
        return 2
    return 0


def _rope_tables_half(pos, base):
    """pos: [n] ints -> cos [128, n], sin [128, n]; row k = freq k."""
    freq = np.power(float(base), -2.0 * np.arange(DG // 2, dtype=np.float64) / DG)
    ang = freq[:, None] * pos[None, :].astype(np.float64)
    return np.cos(ang), np.sin(ang)


def _perm_cols(side_off):
    """New feature order: c*512 + g*256 + p*128 + k  <-  orig
    c*1024 + side_off + g*256 + 2k + p."""
    cols = np.empty(C * D2, np.int64)
    f = 0
    for c in range(C):
        for g in range(2):
            for p in range(2):
                base = c * 1024 + side_off + g * 256 + p
                cols[f:f + 128] = base + 2 * np.arange(128)
                f += 128
    return cols


def _pack_w(Wside):
    """[768, 1536] -> [128, 12, 6, 128] bf16 (ft-major per partition)."""
    w = Wside.reshape(KH, 128, FT, 128).transpose(1, 2, 0, 3)
    return np.ascontiguousarray(w.astype(BF16))


def _tab(pos_tok, pos_utt, sig):
    """[n] positions + sig [2(variant), n] -> [128, 2(g), 3, n] bf16.

    [:, g, 0, :] = cos; [:, g, 1+v, :] = sin * sig[v]."""
    n = pos_tok.shape[0]
    out = np.empty((128, 2, 3, n), np.float64)
    for g, (pos, base) in enumerate(((pos_tok, 10000.0), (pos_utt, 15.0))):
        cos_t, sin_t = _rope_tables_half(pos, base)
        out[:, g, 0] = cos_t
        for v in range(2):
            out[:, g, 1 + v] = sin_t * sig[v][None, :]
    return np.ascontiguousarray(out.astype(BF16))


def _host_prep(x, W1, b1, token_index, utterance_index, seg_ids):
    x = np.asarray(x, np.float32)
    W1 = np.asarray(W1, np.float32)
    b1 = np.asarray(b1, np.float32)
    token_index = np.asarray(token_index)
    utterance_index = np.asarray(utterance_index)
    seg_ids = np.asarray(seg_ids)

    qcols = _perm_cols(0)
    kcols = _perm_cols(512)
    WQp = _pack_w(np.ascontiguousarray(W1[:, qcols]))
    WKp = _pack_w(np.ascontiguousarray(W1[:, kcols]))
    biasc = np.ascontiguousarray(
        np.concatenate([b1[qcols], b1[kcols]]).reshape(2 * FT, 128).T
    ).astype(np.float32)  # [128, 24]

    xTp = []
    for b in range(B):
        xt = np.ascontiguousarray(x[b].T)            # [768, 2048]
        xTp.append(np.ascontiguousarray(
            xt.reshape(KH, 128, S).transpose(1, 0, 2).astype(BF16)))

    in_maps, metas = [], []
    for core in range(N_CORES):
        b, rh, ch = core // 4, (core // 2) % 2, core % 2
        rows = slice(rh * RROWS, (rh + 1) * RROWS)
        cols = slice(ch * CCOLS, (ch + 1) * CCOLS)
        seg = seg_ids[b]

        s_half = np.empty(NH, np.int64)
        for h in range(NH):
            sv = seg[rh * RROWS + h * BLK: rh * RROWS + (h + 1) * BLK]
            if not np.all(sv == sv[0]):
                raise NotImplementedError("fast path: 512-row half must share one seg")
            s_half[h] = sv[0]
        t_blk = np.empty(NB, np.int64)
        for nb in range(NB):
            tv = seg[ch * CCOLS + nb * BLK: ch * CCOLS + (nb + 1) * BLK]
            if not np.all(tv == tv[0]):
                raise NotImplementedError("fast path: 512-col block must share one seg")
            t_blk[nb] = tv[0]

        sigq = np.empty((NH, NB), np.float64)
        sigk = np.empty((NH, NB), np.float64)
        for h in range(NH):
            for nb in range(NB):
                v = _variant(int(s_half[h]), int(t_blk[nb]))
                sigq[h, nb] = -1.0 if v == 1 else 1.0
                sigk[h, nb] = -1.0 if v == 2 else 1.0

        # Q tables over rows; variant v = col-block nb: sign per row = sigma_q(half(row), nb)
        sig_q_rows = np.empty((2, RROWS))
        for v in range(NB):
            for h in range(NH):
                sig_q_rows[v, h * BLK:(h + 1) * BLK] = sigq[h, v]
        # K tables over cols; variant v = row-half h: sign per col = sigma_k(h, blk(col))
        sig_k_cols = np.empty((2, CCOLS))
        for v in range(NH):
            for nb in range(NB):
                sig_k_cols[v, nb * BLK:(nb + 1) * BLK] = sigk[v, nb]

        in_maps.append({
            "XQ": np.ascontiguousarray(xTp[b][:, :, rows]),
            "XK": np.ascontiguousarray(xTp[b][:, :, cols]),
            "WQ": WQp, "WK": WKp, "BIASC": biasc,
            "TABQ": _tab(token_index[b, rows], utterance_index[b, rows], sig_q_rows),
            "TABK": _tab(token_index[b, cols], utterance_index[b, cols], sig_k_cols),
        })
        metas.append({"b": b, "rh": rh, "ch": ch})
    return in_maps, metas


def _build_program(reps=0):
    import concourse.bacc as bacc
    import concourse.mybir as mybir
    import concourse.tile as tile
    from contextlib import ExitStack

    f32 = mybir.dt.float32
    bf16 = mybir.dt.bfloat16
    AF = mybir.ActivationFunctionType
    OP = mybir.AluOpType

    nc = bacc.Bacc("TRN2", target_bir_lowering=False, debug=False,
                   num_devices=N_CORES)
    XQd = nc.dram_tensor("XQ", [128, KH, RROWS], bf16, kind="ExternalInput")
    XKd = nc.dram_tensor("XK", [128, KH, CCOLS], bf16, kind="ExternalInput")
    WQd = nc.dram_tensor("WQ", [128, FT, KH, 128], bf16, kind="ExternalInput")
    WKd = nc.dram_tensor("WK", [128, FT, KH, 128], bf16, kind="ExternalInput")
    BIASC = nc.dram_tensor("BIASC", [128, 2 * FT], f32, kind="ExternalInput")
    TABQ = nc.dram_tensor("TABQ", [128, 2, 3, RROWS], bf16, kind="ExternalInput")
    TABK = nc.dram_tensor("TABK", [128, 2, 3, CCOLS], bf16, kind="ExternalInput")
    OUT = nc.dram_tensor("OUT", [C, RROWS, CCOLS], bf16, kind="ExternalOutput")

    with tile.TileContext(nc) as tc, ExitStack() as ctx:
        constp = ctx.enter_context(tc.tile_pool(name="constp", bufs=1))
        wp = ctx.enter_context(tc.tile_pool(name="wp", bufs=4))
        yp = ctx.enter_context(tc.tile_pool(name="yp", bufs=4))
        prodp = ctx.enter_context(tc.tile_pool(name="prodp", bufs=13))
        effp = ctx.enter_context(tc.tile_pool(name="effp", bufs=36))
        outp = ctx.enter_context(tc.tile_pool(name="outp", bufs=6))
        pap = ctx.enter_context(tc.tile_pool(name="pap", bufs=5, space="PSUM"))
        pbp = ctx.enter_context(tc.tile_pool(name="pbp", bufs=3, space="PSUM"))

        mm = nc.tensor.matmul

        bias_all = constp.tile([128, 2 * FT], f32, name="bias_all")
        nc.sync.dma_start(bias_all[:], BIASC[:])
        # prime the Identity activation table outside the loop so the
        # 1.3us LoadActFuncSet is not paid on every For_i iteration
        warm = constp.tile([128, 1], f32, name="warm")
        nc.scalar.activation(warm[:], bias_all[:, 0:1], AF.Identity)
        tabq_sb = constp.tile([128, 2, 3, RROWS], bf16, name="tabq")
        tabk_sb = constp.tile([128, 2, 3, CCOLS], bf16, name="tabk")
        xq_sb = constp.tile([128, KH, RROWS], bf16, name="xq")
        xk_sb = constp.tile([128, KH, CCOLS], bf16, name="xk")

        def emit_body():
            qeff, keff = {}, {}
            pre_wt = {}
            wtiles = {}

            def load_w(is_k, c):
                # one batched DMA per (side, channel): [128, 4, KH, 128]
                wd = WKd if is_k else WQd
                wt = wp.tile([128, 4, KH, 128], bf16, name="wtb", tag="wtb")
                nc.sync.dma_start(wt[:], wd[:, c * 4:(c + 1) * 4])
                wtiles[(is_k, c)] = wt

            def load_wt(is_k, c, g, p):
                ft = c * 4 + g * 2 + p
                wd = WKd if is_k else WQd
                wt = wp.tile([128, KH, 128], bf16, name="wt", tag="wt")
                nc.sync.dma_start(wt[:], wd[:, ft])
                return wt

            def emit_ft(is_k, c, g, p):
                ftl = g * 2 + p
                xs = xk_sb if is_k else xq_sb
                wt = pre_wt.pop((is_k, c, g, p), None)
                wsl = wt if wt is not None else wtiles[(is_k, c)][:, ftl]
                y = yp.tile([128, RROWS], bf16, name="y", tag="y")
                bi = (FT if is_k else 0) + c * 4 + ftl
                for half in range(2):
                    ps = pap.tile([128, BLK], f32, name="psa")
                    for kh in range(KH):
                        mm(ps[:], wsl[:, kh, :],
                           xs[:, kh, half * BLK:(half + 1) * BLK],
                           start=(kh == 0), stop=(kh == KH - 1))
                    nc.scalar.activation(
                        y[:, half * BLK:(half + 1) * BLK], ps[:], AF.Identity,
                        bias=bias_all[:, bi:bi + 1])
                return y

            def emit_pair(is_k, c, g):
                y_e = emit_ft(is_k, c, g, 0)
                y_o = emit_ft(is_k, c, g, 1)
                tab = tabk_sb if is_k else tabq_sb
                cosg = tab[:, g, 0, :]
                T = lambda nm: prodp.tile([128, RROWS], bf16, name=nm, tag="prod")
                ae, ao = T("ae"), T("ao")
                nc.vector.tensor_tensor(ae[:], y_e[:], cosg, OP.mult)
                nc.vector.tensor_tensor(ao[:], y_o[:], cosg, OP.mult)
                dst = keff if is_k else qeff
                for v in range(2):
                    sing = tab[:, g, 1 + v, :]
                    as_v, bo_v = T("as_v"), T("bo_v")
                    nc.vector.tensor_tensor(as_v[:], y_o[:], sing, OP.mult)
                    nc.vector.tensor_tensor(bo_v[:], y_e[:], sing, OP.mult)
                    e_t = effp.tile([128, RROWS], bf16, name="eff", tag="eff")
                    o_t = effp.tile([128, RROWS], bf16, name="eff", tag="eff")
                    # K-side h=1 effs are consumed only from m=4 of stage B,
                    # so they tolerate the slower Pool engine; Q-side nb=1
                    # effs are needed at m=0 and stay on DVE.
                    eng = nc.gpsimd if (is_k and v == 1) else nc.vector
                    eng.tensor_tensor(e_t[:], ae[:], as_v[:], OP.subtract)
                    eng.tensor_tensor(o_t[:], ao[:], bo_v[:], OP.add)
                    dst[(c, g, 0, v)] = e_t
                    dst[(c, g, 1, v)] = o_t

            def emit_a_side(is_k, c):
                for g in range(2):
                    emit_pair(is_k, c, g)

            def emit_b(c):
                for m in range(MT):
                    h, j = m // 4, m % 4
                    pss = [pbp.tile([128, BLK], f32, name="psb")
                           for _ in range(NB)]
                    for dt in range(4):
                        g, p = dt // 2, dt % 2
                        for nb in range(NB):
                            mm(pss[nb][:],
                               qeff[(c, g, p, nb)][:, m * 128:(m + 1) * 128],
                               keff[(c, g, p, h)][:, nb * BLK:(nb + 1) * BLK],
                               start=(dt == 0), stop=(dt == 3))
                    ob = outp.tile([128, CCOLS], bf16, name="ob", tag="ob")
                    for nb in range(NB):
                        nc.scalar.activation(
                            ob[:, nb * BLK:(nb + 1) * BLK], pss[nb][:], AF.Copy)
                    nc.scalar.dma_start(OUT[c, m * 128:(m + 1) * 128, :], ob[:])

            # Every tile's DMA is emitted before its readers (Tile needs
            # program-order producer->consumer).  Startup is hidden by ring
            # split: the sync (SP) HWDGE ring feeds the first K chains
            # (wt + xk interleaved), while xq and the rope tables stream
            # concurrently on the scalar (ACT) HWDGE ring.
            pre_wt[(True, 0, 0, 0)] = load_wt(True, 0, 0, 0)
            nc.sync.dma_start(xk_sb[:, 0:3, :], XKd[:, 0:3, :])
            pre_wt[(True, 0, 0, 1)] = load_wt(True, 0, 0, 1)
            pre_wt[(True, 0, 1, 0)] = load_wt(True, 0, 1, 0)
            pre_wt[(True, 0, 1, 1)] = load_wt(True, 0, 1, 1)
            nc.sync.dma_start(xk_sb[:, 3:KH, :], XKd[:, 3:KH, :])
            load_w(True, 1)
            nc.sync.dma_start(tabk_sb[:], TABK[:])
            # K-sides first: the Q-side x/table streams get ~20us of DMA
            # slack before their first consumers.
            emit_a_side(True, 0)
            nc.sync.dma_start(xq_sb[:, 0:3, :], XQd[:, 0:3, :])
            load_w(False, 0)
            emit_a_side(True, 1)
            nc.sync.dma_start(xq_sb[:, 3:KH, :], XQd[:, 3:KH, :])
            nc.sync.dma_start(tabq_sb[:], TABQ[:])
            load_w(False, 1)
            emit_a_side(False, 0)
            load_w(True, 2)
            emit_a_side(False, 1)
            load_w(False, 2)
            emit_b(0)
            emit_a_side(True, 2)
            emit_a_side(False, 2)
            emit_b(1)
            emit_b(2)

        if reps and reps > 1:
            with tc.For_i(0, reps, 1):
                emit_body()
        else:
            emit_body()

    nc.compile()
    return nc


_PROG_CACHE = {}


def kernel(**inputs):
    from concourse.bass_utils import run_bass_kernel_spmd

    in_maps, metas = _host_prep(**inputs)
    if "prog" not in _PROG_CACHE:
        _PROG_CACHE["prog"] = _build_program()
    nc = _PROG_CACHE["prog"]

    res = run_bass_kernel_spmd(nc, in_maps, list(range(N_CORES)))
    out = np.empty((B, S, S, C), np.float32)
    for core in range(N_CORES):
        m = metas[core]
        o = np.asarray(res.results[core]["OUT"], np.float32)  # [C, 1024, 1024]
        out[m["b"],
            m["rh"] * RROWS:(m["rh"] + 1) * RROWS,
            m["ch"] * CCOLS:(m["ch"] + 1) * CCOLS] = o.transpose(1, 2, 0)
    return out


# revision 3
# speedup vs baseline: 1.0375x; 1.0375x over previous
"""Trainium2 Bass kernel for nn_BertWordPair (sparse_attention) — v2.

y = x @ W1 + b1 -> (q_tok, q_utt, k_tok, k_utt) per channel c; RoPE with
block-sign structure from seg_ids; logits [B, S, S, 3] = selected-variant
bilinear forms summed over the two groups.

Strategy (8 NeuronCores), v2:
  - 2x2 grid per batch: core (b, rh, ch) owns rows rh*1024..+1024 and cols
    ch*1024..+1024.  Stage-A computes q-features for the core's rows and
    k-features for the core's cols only (147K PE cycles vs 184K for 1x4).
  - bf16 datapath except PSUM accumulation: W/x/tables load bf16, stage-A
    PSUM (fp32) is bias-added + cast to bf16 on the Activation engine, all
    RoPE elementwise runs as bf16 tensor_tensor (2x DVE mode), stage-B
    matmuls take bf16 operands, output written bf16 and upcast on host.
  - Variant selection (pp / q_neg.k_pos / q_pos.k_neg) reduces to signs
    sigma_q(row-half, col-block) / sigma_k(row-half, col-block).  The signs
    are folded into per-variant HOST-built sin tables so the SPMD program
    stays uniform and sign application costs nothing:
      K side: sin_v[h][:, col] = sin[:, col] * sigma_k(h, blk(col))
      Q side: sin_v[nb][:, row] = sin[:, row] * sigma_q(half(row), nb)
    K_eff^h_e = y_e*cos - y_o*sin_v[h]   (plain TT sub/add, no scalars)
"""
import sys
sys.path.insert(0, '/opt/trn_rl_repo')

import numpy as np
import ml_dtypes

BF16 = ml_dtypes.bfloat16

B, S, H, C = 2, 2048, 768, 3
DG = 256             # rope dim per group (tok / utt)
D2 = 512             # feature dim per channel (tok 256 + utt 256)
N_CORES = 8
RROWS = 1024         # rows per core
CCOLS = 1024         # cols per core
BLK = 512            # sigma granularity (col block / row half)
NB = CCOLS // BLK    # 2
NH = RROWS // BLK    # 2
KH = H // 128        # 6 contraction tiles for dense1
FT = 12              # feature tiles per side (q or k): ft = c*4 + g*2 + p
MT = RROWS // 128    # 8 row tiles per core


def _variant(s, t):
    # 0=PP, 1=NP (q_neg*k_pos), 2=PN (q_pos*k_neg)
    if s >= 1 and t > s:
        return 1
    if t >= 1 and s > t:
        return 2
    return 0


def _rope_tables_half(pos, base):
    """pos: [n] ints -> cos [128, n], sin [128, n]; row k = freq k."""
    freq = np.power(float(base), -2.0 * np.arange(DG // 2, dtype=np.float64) / DG)
    ang = freq[:, None] * pos[None, :].astype(np.float64)
    return np.cos(ang), np.sin(ang)


def _perm_cols(side_off):
    """New feature order: c*512 + g*256 + p*128 + k  <-  orig
    c*1024 + side_off + g*256 + 2k + p."""
    cols = np.empty(C * D2, np.int64)
    f = 0
    for c in range(C):
        for g in range(2):
            for p in range(2):
                base = c * 1024 + side_off + g * 256 + p
                cols[f:f + 128] = base + 2 * np.arange(128)
                f += 128
    return cols


def _pack_w(Wside):
    """[768, 1536] -> [128, 12, 6, 128] bf16 (ft-major per partition)."""
    w = Wside.reshape(KH, 128, FT, 128).transpose(1, 2, 0, 3)
    return np.ascontiguousarray(w.astype(BF16))


def _tab(pos_tok, pos_utt, sig):
    """[n] positions + sig [2(variant), n] -> [128, 2(g), 3, n] bf16.

    [:, g, 0, :] = cos; [:, g, 1+v, :] = sin * sig[v]."""
    n = pos_tok.shape[0]
    out = np.empty((128, 2, 3, n), np.float64)
    for g, (pos, base) in enumerate(((pos_tok, 10000.0), (pos_utt, 15.0))):
        cos_t, sin_t = _rope_tables_half(pos, base)
        out[:, g, 0] = cos_t
        for v in range(2):
            out[:, g, 1 + v] = sin_t * sig[v][None, :]
    return np.ascontiguousarray(out.astype(BF16))


def _host_prep(x, W1, b1, token_index, utterance_index, seg_ids):
    x = np.asarray(x, np.float32)
    W1 = np.asarray(W1, np.float32)
    b1 = np.asarray(b1, np.float32)
    token_index = np.asarray(token_index)
    utterance_index = np.asarray(utterance_index)
    seg_ids = np.asarray(seg_ids)

    qcols = _perm_cols(0)
    kcols = _perm_cols(512)
    WQp = _pack_w(np.ascontiguousarray(W1[:, qcols]))
    WKp = _pack_w(np.ascontiguousarray(W1[:, kcols]))
    biasc = np.ascontiguousarray(
        np.concatenate([b1[qcols], b1[kcols]]).reshape(2 * FT, 128).T
    ).astype(np.float32)  # [128, 24]

    xTp = []
    for b in range(B):
        xt = np.ascontiguousarray(x[b].T)            # [768, 2048]
        xTp.append(np.ascontiguousarray(
            xt.reshape(KH, 128, S).transpose(1, 0, 2).astype(BF16)))

    in_maps, metas = [], []
    for core in range(N_CORES):
        b, rh, ch = core // 4, (core // 2) % 2, core % 2
        rows = slice(rh * RROWS, (rh + 1) * RROWS)
        cols = slice(ch * CCOLS, (ch + 1) * CCOLS)
        seg = seg_ids[b]

        s_half = np.empty(NH, np.int64)
        for h in range(NH):
            sv = seg[rh * RROWS + h * BLK: rh * RROWS + (h + 1) * BLK]
            if not np.all(sv == sv[0]):
                raise NotImplementedError("fast path: 512-row half must share one seg")
            s_half[h] = sv[0]
        t_blk = np.empty(NB, np.int64)
        for nb in range(NB):
            tv = seg[ch * CCOLS + nb * BLK: ch * CCOLS + (nb + 1) * BLK]
            if not np.all(tv == tv[0]):
                raise NotImplementedError("fast path: 512-col block must share one seg")
            t_blk[nb] = tv[0]

        sigq = np.empty((NH, NB), np.float64)
        sigk = np.empty((NH, NB), np.float64)
        for h in range(NH):
            for nb in range(NB):
                v = _variant(int(s_half[h]), int(t_blk[nb]))
                sigq[h, nb] = -1.0 if v == 1 else 1.0
                sigk[h, nb] = -1.0 if v == 2 else 1.0

        # Q tables over rows; variant v = col-block nb: sign per row = sigma_q(half(row), nb)
        sig_q_rows = np.empty((2, RROWS))
        for v in range(NB):
            for h in range(NH):
                sig_q_rows[v, h * BLK:(h + 1) * BLK] = sigq[h, v]
        # K tables over cols; variant v = row-half h: sign per col = sigma_k(h, blk(col))
        sig_k_cols = np.empty((2, CCOLS))
        for v in range(NH):
            for nb in range(NB):
                sig_k_cols[v, nb * BLK:(nb + 1) * BLK] = sigk[v, nb]

        in_maps.append({
            "XQ": np.ascontiguousarray(xTp[b][:, :, rows]),
            "XK": np.ascontiguousarray(xTp[b][:, :, cols]),
            "WQ": WQp, "WK": WKp, "BIASC": biasc,
            "TABQ": _tab(token_index[b, rows], utterance_index[b, rows], sig_q_rows),
            "TABK": _tab(token_index[b, cols], utterance_index[b, cols], sig_k_cols),
        })
        metas.append({"b": b, "rh": rh, "ch": ch})
    return in_maps, metas


def _build_program(reps=0):
    import concourse.bacc as bacc
    import concourse.mybir as mybir
    import concourse.tile as tile
    from contextlib import ExitStack

    f32 = mybir.dt.float32
    bf16 = mybir.dt.bfloat16
    AF = mybir.ActivationFunctionType
    OP = mybir.AluOpType

    nc = bacc.Bacc("TRN2", target_bir_lowering=False, debug=False,
                   num_devices=N_CORES)
    XQd = nc.dram_tensor("XQ", [128, KH, RROWS], bf16, kind="ExternalInput")
    XKd = nc.dram_tensor("XK", [128, KH, CCOLS], bf16, kind="ExternalInput")
    WQd = nc.dram_tensor("WQ", [128, FT, KH, 128], bf16, kind="ExternalInput")
    WKd = nc.dram_tensor("WK", [128, FT, KH, 128], bf16, kind="ExternalInput")
    BIASC = nc.dram_tensor("BIASC", [128, 2 * FT], f32, kind="ExternalInput")
    TABQ = nc.dram_tensor("TABQ", [128, 2, 3, RROWS], bf16, kind="ExternalInput")
    TABK = nc.dram_tensor("TABK", [128, 2, 3, CCOLS], bf16, kind="ExternalInput")
    OUT = nc.dram_tensor("OUT", [C, RROWS, CCOLS], bf16, kind="ExternalOutput")

    with tile.TileContext(nc) as tc, ExitStack() as ctx:
        constp = ctx.enter_context(tc.tile_pool(name="constp", bufs=1))
        wp = ctx.enter_context(tc.tile_pool(name="wp", bufs=4))
        yp = ctx.enter_context(tc.tile_pool(name="yp", bufs=4))
        prodp = ctx.enter_context(tc.tile_pool(name="prodp", bufs=5))
        effp = ctx.enter_context(tc.tile_pool(name="effp", bufs=36))
        outp = ctx.enter_context(tc.tile_pool(name="outp", bufs=6))
        pap = ctx.enter_context(tc.tile_pool(name="pap", bufs=4, space="PSUM"))
        pbp = ctx.enter_context(tc.tile_pool(name="pbp", bufs=4, space="PSUM"))

        mm = nc.tensor.matmul

        bias_all = constp.tile([128, 2 * FT], f32, name="bias_all")
        nc.sync.dma_start(bias_all[:], BIASC[:])
        # prime the Identity activation table outside the loop so the
        # 1.3us LoadActFuncSet is not paid on every For_i iteration
        warm = constp.tile([128, 1], f32, name="warm")
        nc.scalar.activation(warm[:], bias_all[:, 0:1], AF.Identity)
        tabq_sb = constp.tile([128, 2, 3, RROWS], bf16, name="tabq")
        tabk_sb = constp.tile([128, 2, 3, CCOLS], bf16, name="tabk")
        xq_sb = constp.tile([128, KH, RROWS], bf16, name="xq")
        xk_sb = constp.tile([128, KH, CCOLS], bf16, name="xk")

        def emit_body():
            qeff, keff = {}, {}
            pre_wt = {}
            wtiles = {}

            def load_w(is_k, c):
                # one batched DMA per (side, channel): [128, 4, KH, 128]
                wd = WKd if is_k else WQd
                wt = wp.tile([128, 4, KH, 128], bf16, name="wtb", tag="wtb")
                nc.sync.dma_start(wt[:], wd[:, c * 4:(c + 1) * 4])
                wtiles[(is_k, c)] = wt

            def load_wt(is_k, c, g, p):
                ft = c * 4 + g * 2 + p
                wd = WKd if is_k else WQd
                wt = wp.tile([128, KH, 128], bf16, name="wt", tag="wt")
                nc.sync.dma_start(wt[:], wd[:, ft])
                return wt

            def emit_ft(is_k, c, g, p):
                ftl = g * 2 + p
                xs = xk_sb if is_k else xq_sb
                wt = pre_wt.pop((is_k, c, g, p), None)
                wsl = wt if wt is not None else wtiles[(is_k, c)][:, ftl]
                y = yp.tile([128, RROWS], bf16, name="y", tag="y")
                bi = (FT if is_k else 0) + c * 4 + ftl
                for half in range(2):
                    ps = pap.tile([128, BLK], f32, name="psa")
                    for kh in range(KH):
                        mm(ps[:], wsl[:, kh, :],
                           xs[:, kh, half * BLK:(half + 1) * BLK],
                           start=(kh == 0), stop=(kh == KH - 1))
                    nc.scalar.activation(
                        y[:, half * BLK:(half + 1) * BLK], ps[:], AF.Identity,
                        bias=bias_all[:, bi:bi + 1])
                return y

            def emit_pair(is_k, c, g):
                y_e = emit_ft(is_k, c, g, 0)
                y_o = emit_ft(is_k, c, g, 1)
                tab = tabk_sb if is_k else tabq_sb
                # fused products: one broadcast TT per parity computes
                # {cos, sin_v0, sin_v1} * y in a single [128, 3, n] op
                pe = prodp.tile([128, 3, RROWS], bf16, name="pe", tag="prod")
                po = prodp.tile([128, 3, RROWS], bf16, name="po", tag="prod")
                tslice = tab[:, g, :, :]
                nc.vector.tensor_tensor(
                    pe[:], y_e[:].unsqueeze(1).broadcast_to([128, 3, RROWS]),
                    tslice, OP.mult)
                nc.vector.tensor_tensor(
                    po[:], y_o[:].unsqueeze(1).broadcast_to([128, 3, RROWS]),
                    tslice, OP.mult)
                dst = keff if is_k else qeff
                for v in range(2):
                    e_t = effp.tile([128, RROWS], bf16, name="eff", tag="eff")
                    o_t = effp.tile([128, RROWS], bf16, name="eff", tag="eff")
                    # K-side h=1 effs are consumed only from m=4 of stage B,
                    # so they tolerate the slower Pool engine; Q-side nb=1
                    # effs are needed at m=0 and stay on DVE.
                    eng = nc.gpsimd if (is_k and v == 1) else nc.vector
                    eng.tensor_tensor(e_t[:], pe[:, 0, :], po[:, 1 + v, :],
                                      OP.subtract)
                    eng.tensor_tensor(o_t[:], po[:, 0, :], pe[:, 1 + v, :],
                                      OP.add)
                    dst[(c, g, 0, v)] = e_t
                    dst[(c, g, 1, v)] = o_t

            def emit_a_side(is_k, c):
                for g in range(2):
                    emit_pair(is_k, c, g)

            def emit_b(c):
                for m in range(MT):
                    h, j = m // 4, m % 4
                    pss = [pbp.tile([128, BLK], f32, name="psb")
                           for _ in range(NB)]
                    for dt in range(4):
                        g, p = dt // 2, dt % 2
                        for nb in range(NB):
                            mm(pss[nb][:],
                               qeff[(c, g, p, nb)][:, m * 128:(m + 1) * 128],
                               keff[(c, g, p, h)][:, nb * BLK:(nb + 1) * BLK],
                               start=(dt == 0), stop=(dt == 3))
                    ob = outp.tile([128, CCOLS], bf16, name="ob", tag="ob")
                    for nb in range(NB):
                        nc.scalar.activation(
                            ob[:, nb * BLK:(nb + 1) * BLK], pss[nb][:], AF.Copy)
                    nc.scalar.dma_start(OUT[c, m * 128:(m + 1) * 128, :], ob[:])

            # Every tile's DMA is emitted before its readers (Tile needs
            # program-order producer->consumer).  Startup is hidden by ring
            # split: the sync (SP) HWDGE ring feeds the first K chains
            # (wt + xk interleaved), while xq and the rope tables stream
            # concurrently on the scalar (ACT) HWDGE ring.
            pre_wt[(True, 0, 0, 0)] = load_wt(True, 0, 0, 0)
            nc.sync.dma_start(xk_sb[:, 0, :], XKd[:, 0, :])
            nc.sync.dma_start(xk_sb[:, 1:3, :], XKd[:, 1:3, :])
            pre_wt[(True, 0, 0, 1)] = load_wt(True, 0, 0, 1)
            pre_wt[(True, 0, 1, 0)] = load_wt(True, 0, 1, 0)
            pre_wt[(True, 0, 1, 1)] = load_wt(True, 0, 1, 1)
            nc.sync.dma_start(xk_sb[:, 3:KH, :], XKd[:, 3:KH, :])
            load_w(True, 1)
            nc.sync.dma_start(tabk_sb[:], TABK[:])
            # K-sides first: the Q-side x/table streams get ~20us of DMA
            # slack before their first consumers.
            emit_a_side(True, 0)
            nc.sync.dma_start(xq_sb[:, 0:3, :], XQd[:, 0:3, :])
            load_w(False, 0)
            emit_a_side(True, 1)
            nc.sync.dma_start(xq_sb[:, 3:KH, :], XQd[:, 3:KH, :])
            nc.sync.dma_start(tabq_sb[:], TABQ[:])
            load_w(False, 1)
            emit_a_side(False, 0)
            load_w(True, 2)
            emit_a_side(False, 1)
            load_w(False, 2)
            emit_b(0)
            emit_a_side(True, 2)
            emit_a_side(False, 2)
            emit_b(1)
            emit_b(2)

        if reps and reps > 1:
            with tc.For_i(0, reps, 1):
                emit_body()
        else:
            emit_body()

    nc.compile()
    return nc


_PROG_CACHE = {}


def kernel(**inputs):
    from concourse.bass_utils import run_bass_kernel_spmd

    in_maps, metas = _host_prep(**inputs)
    if "prog" not in _PROG_CACHE:
        _PROG_CACHE["prog"] = _build_program()
    nc = _PROG_CACHE["prog"]

    res = run_bass_kernel_spmd(nc, in_maps, list(range(N_CORES)))
    out = np.empty((B, S, S, C), np.float32)
    for core in range(N_CORES):
        m = metas[core]
        o = np.asarray(res.results[core]["OUT"], np.float32)  # [C, 1024, 1024]
        out[m["b"],
            m["rh"] * RROWS:(m["rh"] + 1) * RROWS,
            m["ch"] * CCOLS:(m["ch"] + 1) * CCOLS] = o.transpose(1, 2, 0)
    return out
